# revision 6
# baseline (speedup 1.0000x reference)
"""CISS-VAE (per-cluster MoE-routed MLP chain) Trainium2 kernel.

Strategy (routing on host, compute on device):
  - Rows are grouped by cluster label on the host. Core c processes all rows
    of cluster c (C == n_cores == 8), so every GEMM on the device is a dense
    per-cluster GEMM.
  - The encoder (enc0, encu, enc2, mu, lv) runs in fp8-e4m3 with DoubleRow
    perf mode (2x PE throughput). This is numerically safe because the VAE
    latent z = mu + exp(0.5*logvar)*eps is dominated by the exact eps input:
    quantizing the whole encoder to fp8 moves the final output by ~2e-4
    relative (measured against the f32 reference on the real input stats).
  - The decoder (dec0, dec1, dec2, fin) stays bf16: fp8 there costs 1-2e-2
    relative error, too close to the tolerance.
  - fp8 operands are pre-scaled by powers of two (exact): x by 2^4, encoder
    weights by 2^8, hidden activations by 2^4. The descale (2^-8 per layer)
    is folded into the PSUM eviction's activation scale.
  - All tensors on device are feature-major; k-tiles are packed in the free
    dimension ([128, n_ktiles, rows]) so a DoubleRow matmul can consume two
    k-tiles per instruction via a 3-d access pattern.
  - Per-feature biases live on partitions and are fused into the PSUM->SBUF
    eviction (Relu/Identity/Exp). The encoder needs scale+bias+relu (3 alu
    stages) so its evictions go to the Scalar engine; decoder evictions
    alternate Scalar/Vector.
  - The encoder is eviction-bound (a PSUM group every ~220ns vs ~650ns per
    Scalar eviction), so encoder m-tile groups of block b are WOVEN with
    decoder groups of block b-1 at group granularity: the decoder's long
    bf16 matmul groups give the Scalar engine time to drain encoder PSUMs,
    and the PSUM pool never backs up.
  - x / out DRAM layouts are block-contiguous so the per-block DMAs are
    single contiguous transfers on the sync (HWDGE) queue.
"""

import ml_dtypes
import numpy as np

import concourse.bacc as bacc
import concourse.mybir as mybir
import concourse.tile as tile
from concourse import bass_utils

P = 128
D_IN, LAT, C = 512, 64, 8
H0, H1, H2 = 1024, 512, 256
N_CORES = 8
KX = D_IN // P
F32 = mybir.dt.float32
BF16 = mybir.dt.bfloat16
FP8 = mybir.dt.float8e4
AF = mybir.ActivationFunctionType
ALU = mybir.AluOpType
DR = mybir.MatmulPerfMode.DoubleRow
BF16_NP = ml_dtypes.bfloat16
FP8_NP = ml_dtypes.float8_e4m3

SX = 16.0    # fp8 scale on x
SW = 256.0   # fp8 scale on encoder weights
SA = 16.0    # fp8 scale on hidden activations
SEV = SA / (SX * SW)   # eviction scale for fp8 relu layers (2^-8)

# layer table: name -> (f_in, f_out, fp8)
LAYERS = dict(
    enc0=(D_IN, H0, True),
    encu=(H0, H1, True),
    enc2=(H1, H2, True),
    mu=(H2, LAT, True),
    lv=(H2, LAT, True),
    dec0=(LAT, H2, False),
    dec1=(H2, H1, False),
    dec2=(H1, H0, False),
    fin=(H0, D_IN, False),
)


def _ceil_to(x, m):
    return ((x + m - 1) // m) * m


def _b2d(b, scale=1.0):
    """[f] bias -> [min(f,128), n_mtiles] (partition-major per m-tile)."""
    b = np.asarray(b, dtype=np.float32) * scale
    f = b.shape[0]
    if f >= P:
        return np.ascontiguousarray(b.reshape(f // P, P).T.astype(np.float32))
    return np.ascontiguousarray(b.reshape(1, f).T.astype(np.float32))


def _wpack(W, dt_np, scale=1.0):
    """[fi, fo] -> [min(fi,128), kt*fo] k-tile packed, cast to dt_np."""
    W = np.asarray(W, dtype=np.float32) * scale
    fi, fo = W.shape
    kp = min(P, fi)
    kt = max(1, fi // P)
    Wp = W.reshape(kt, kp, fo).transpose(1, 0, 2)
    Wp = np.clip(Wp, -240, 240).astype(dt_np)
    return np.ascontiguousarray(Wp.reshape(kp, kt * fo))


def _weave(*lists):
    """Merge lists of closures, interleaving by fractional progress."""
    lists = [l for l in lists if l]
    idx = [0] * len(lists)
    out = []
    while True:
        best, bp = -1, 2.0
        for j, l in enumerate(lists):
            if idx[j] < len(l):
                p = idx[j] / len(l)
                if p < bp:
                    best, bp = j, p
        if best < 0:
            return out
        out.append(lists[best][idx[best]])
        idx[best] += 1


def _build_module(npad, blocks):
    nc = bacc.Bacc("TRN2", target_bir_lowering=False, debug=False)

    dram = {}

    def din(name, shape, dt):
        dram[name] = nc.dram_tensor(name, list(shape), dt, kind="ExternalInput").ap()
        return dram[name]

    xT = din("xT", (P, KX * npad), FP8)
    epsT = din("epsT", (LAT, npad), F32)

    for name, (fi, fo, fp8) in LAYERS.items():
        kp = min(P, fi)
        kt = max(1, fi // P)
        din("w_" + name, (kp, kt * fo), FP8 if fp8 else BF16)
        din("b_" + name, (P if fo >= P else fo, max(1, fo // P)), F32)

    outT = nc.dram_tensor("outT", [P, KX * npad], F32, kind="ExternalOutput").ap()

    with tile.TileContext(nc) as tc:
        with (
            tc.tile_pool(name="wpool", bufs=1) as wpool,
            tc.tile_pool(name="acts", bufs=2) as acts,
            tc.tile_pool(name="psum", bufs=7, space="PSUM") as psum,
        ):
            wsb = {}
            bsb = {}
            dma_rr = [0]

            def prologue_dma(out, in_):
                eng = nc.sync if dma_rr[0] % 2 == 0 else nc.scalar
                dma_rr[0] += 1
                eng.dma_start(out, in_)

            DEC_W = ("dec0", "dec1", "dec2", "fin")

            def load_weights(name):
                if name in wsb:
                    return
                fi, fo, fp8 = LAYERS[name]
                kp = min(P, fi)
                kt = max(1, fi // P)
                dt = FP8 if fp8 else BF16
                w_t = wpool.tile([kp, kt, fo], dt, tag=f"w_{name}", name=f"w_{name}")
                src = dram["w_" + name].rearrange("p (k f) -> p k f", k=kt)
                if name == "enc0":
                    prologue_dma(w_t[:], src)
                elif name in DEC_W:
                    nc.gpsimd.dma_start(w_t[:], src)
                else:
                    nc.sync.dma_start(w_t[:], src)
                bp = P if fo >= P else fo
                b_t = wpool.tile([bp, max(1, fo // P)], F32, tag=f"b_{name}", name=f"b_{name}")
                nc.gpsimd.dma_start(b_t[:], dram["b_" + name][:])
                wsb[name] = w_t
                bsb[name] = b_t

            def groups_fp8(lname, in_t, nb, func, scale, out_dt, evict="act"):
                """DoubleRow fp8 GEMM out = func(scale*(W.T @ in) + b).
                Returns (out_tile, [emit closures, one per m-tile])."""
                load_weights(lname)
                fi, fo, _ = LAYERS[lname]
                w_t, b_t = wsb[lname], bsb[lname]
                kt = fi // P
                n_m = max(1, fo // P)
                mp = min(P, fo)
                nch = _ceil_to(nb, 256) // 256
                csz = nb // nch
                if fo >= P:
                    o_t = acts.tile([P, n_m, nb], out_dt, tag=f"h_{lname}", name=f"h_{lname}")
                else:
                    o_t = acts.tile([fo, nb], out_dt, tag=f"h_{lname}", name=f"h_{lname}")

                def mk(m):
                    def emit():
                        bias = b_t[:mp, m : m + 1]
                        ps = psum.tile([mp, nb], F32, tag="ps", name=f"ps_{lname}_{m}")
                        for c in range(nch):
                            sl = slice(c * csz, (c + 1) * csz)
                            for i in range(kt // 2):
                                nc.tensor.matmul(
                                    ps[:, sl],
                                    w_t[:, 2 * i : 2 * i + 2, m * mp : (m + 1) * mp],
                                    in_t[:, 2 * i : 2 * i + 2, sl],
                                    start=(i == 0),
                                    stop=(i == kt // 2 - 1),
                                    perf_mode=DR,
                                )
                        dst = o_t[:, m, :] if fo >= P else o_t[:]
                        if evict == "act":
                            nc.scalar.activation(dst, ps[:], func, bias=bias, scale=scale)
                        else:  # dve identity: (ps * scale) + bias
                            nc.vector.tensor_scalar(dst, ps[:], scale, bias, ALU.mult, ALU.add)
                    return emit

                return o_t, [mk(m) for m in range(n_m)]

            def groups_bf16(lname, in_t, nb, func, out_dt=BF16, out_tag=None):
                """bf16 GEMM; returns (out_tile, [closures])."""
                load_weights(lname)
                fi, fo, _ = LAYERS[lname]
                w_t, b_t = wsb[lname], bsb[lname]
                kt = max(1, fi // P)
                n_m = max(1, fo // P)
                mp = min(P, fo)
                tag = out_tag or f"h_{lname}"
                o_t = acts.tile([P, n_m, nb], out_dt, tag=tag, name=tag)

                def mk(m):
                    def emit():
                        bias = b_t[:mp, m : m + 1]
                        ps = psum.tile([mp, nb], F32, tag="ps", name=f"ps_{lname}_{m}")
                        for k in range(kt):
                            mov = in_t[:, k, :] if kt > 1 else in_t[:]
                            nc.tensor.matmul(
                                ps[:],
                                w_t[:, k, m * mp : (m + 1) * mp],
                                mov,
                                start=(k == 0),
                                stop=(k == kt - 1),
                            )
                        dst = o_t[:, m, :]
                        if m % 2 == 1:
                            if func is AF.Relu:
                                nc.vector.tensor_scalar(dst, ps[:], bias, 0.0, ALU.add, ALU.max)
                            else:
                                nc.vector.tensor_scalar(dst, ps[:], bias, None, ALU.add)
                        else:
                            nc.scalar.activation(
                                dst, ps[:], func if func is not None else AF.Identity,
                                bias=bias, scale=1.0,
                            )
                    return emit

                return o_t, [mk(m) for m in range(n_m)]

            n_blk = len(blocks)
            offs = [sum(blocks[:i]) for i in range(n_blk)]
            x_in = [None] * n_blk
            eps_in = [None] * n_blk
            mu_sg = [None] * n_blk
            lat_out = [None] * n_blk

            def stage_load(b):
                nb, off = blocks[b], offs[b]
                x_t = acts.tile([P, KX, nb], FP8, tag="x", bufs=3, name="x")
                src = xT[:, KX * off : KX * (off + nb)].rearrange("p (k n) -> p k n", k=KX)
                (prologue_dma if b == 0 else nc.sync.dma_start)(x_t[:], src)
                e_t = acts.tile([LAT, nb], F32, tag="eps", bufs=3, name="e_t")
                (prologue_dma if b == 0 else nc.sync.dma_start)(e_t[:], epsT[:, off : off + nb])
                x_in[b], eps_in[b] = x_t, e_t

            def build_enc(b):
                nb = blocks[b]
                h0, g0 = groups_fp8("enc0", x_in[b], nb, AF.Relu, SEV, FP8)
                h1, g1 = groups_fp8("encu", h0, nb, AF.Relu, SEV, FP8)
                h2, g2 = groups_fp8("enc2", h1, nb, AF.Relu, SEV, FP8)
                mu, gm = groups_fp8("mu", h2, nb, None, 1.0 / (SA * SW), F32, evict="dve")
                sg, gl = groups_fp8("lv", h2, nb, AF.Exp, 0.5 / (SA * SW), F32)
                mu_sg[b] = (mu, sg)
                return g0 + g1 + g2 + gm + gl

            def stage_lat(b):
                nb = blocks[b]
                mu, sg = mu_sg[b]
                tmp = acts.tile([LAT, nb], F32, tag="tmp", bufs=2, name="tmp")
                nc.vector.tensor_mul(tmp[:], sg[:], eps_in[b][:])
                z = acts.tile([LAT, nb], BF16, tag="z", bufs=2, name="z")
                nc.vector.tensor_add(z[:], tmp[:], mu[:])
                lat_out[b] = z

            def build_dec(b):
                nb, off = blocks[b], offs[b]
                h3, g3 = groups_bf16("dec0", lat_out[b], nb, AF.Relu)
                h4, g4 = groups_bf16("dec1", h3, nb, AF.Relu)
                h5, g5 = groups_bf16("dec2", h4, nb, AF.Relu)
                ot, g6 = groups_bf16("fin", h5, nb, None, out_dt=F32, out_tag="out")

                def store():
                    nc.sync.dma_start(
                        outT[:, KX * off : KX * (off + nb)],
                        ot[:].rearrange("p k n -> p (k n)"),
                    )

                return g3 + g4 + g5 + g6 + [store]

            # Warm up the PE (clock gate) with dummy matmuls while the
            # prologue DMAs stream in.
            wu_w = wpool.tile([P, P], BF16, tag="wu_w", name="wu_w")
            wu_x = wpool.tile([P, 512], BF16, tag="wu_x", name="wu_x")
            nc.vector.memset(wu_w[:], 0.0)
            nc.vector.memset(wu_x[:], 0.0)
            wu_ps = psum.tile([P, 512], F32, tag="wu_ps", bufs=1, name="wu_ps")
            for _ in range(20):
                nc.tensor.matmul(wu_ps[:], wu_w[:], wu_x[:], start=True, stop=True)

            # software pipeline: weave encoder groups of block b with decoder
            # groups of block b-1 so decoder matmuls cover encoder evictions.
            stage_load(0)
            for g in build_enc(0):
                g()
            stage_lat(0)
            for b in range(1, n_blk):
                stage_load(b)
                for g in _weave(build_enc(b), build_dec(b - 1)):
                    g()
                stage_lat(b)
            for g in build_dec(n_blk - 1):
                g()

    nc.compile()
    return nc


def kernel(**inputs):
    x = np.asarray(inputs["x"], dtype=np.float32)
    lbl = np.asarray(inputs["cluster_labels"]).astype(np.int64)
    eps = np.asarray(inputs["eps"], dtype=np.float32)
    B = x.shape[0]

    counts = np.bincount(lbl, minlength=C)
    npad = max(512, _ceil_to(int(counts.max()), 128))
    n_full, rem = divmod(npad, 512)
    blocks = [512] * n_full + ([rem] if rem else [])

    rows = [np.nonzero(lbl == c)[0] for c in range(C)]

    shared = {
        "w_enc0": _wpack(inputs["enc_W0"], FP8_NP, SW),
        "b_enc0": _b2d(inputs["enc_b0"], SA),
        "w_enc2": _wpack(inputs["enc_W2"], FP8_NP, SW),
        "b_enc2": _b2d(inputs["enc_b2"], SA),
        "w_mu": _wpack(inputs["mu_W"], FP8_NP, SW),
        "b_mu": _b2d(inputs["mu_b"]),
        "w_lv": _wpack(inputs["lv_W"], FP8_NP, SW),
        "b_lv": _b2d(inputs["lv_b"], 0.5),
        "w_dec1": _wpack(inputs["dec_W1"], BF16_NP),
        "b_dec1": _b2d(inputs["dec_b1"]),
    }

    in_maps = []
    for c in range(C):
        r = rows[c]
        xq = np.zeros((P, KX * npad), FP8_NP)
        xs = np.clip(
            x[r].T.reshape(KX, P, len(r)).transpose(1, 0, 2) * SX, -240, 240
        ).astype(FP8_NP)
        # block-contiguous layout: block b occupies cols [KX*off, KX*(off+nb))
        # with k-tile stride nb inside the block
        for off, nb in zip([sum(blocks[:i]) for i in range(len(blocks))], blocks):
            lo, hi = off, min(off + nb, len(r))
            if lo >= len(r):
                break
            seg = np.zeros((P, KX, nb), FP8_NP)
            seg[:, :, : hi - lo] = xs[:, :, lo:hi]
            xq[:, KX * off : KX * (off + nb)] = seg.reshape(P, -1)
        epsT = np.zeros((LAT, npad), np.float32)
        epsT[:, : len(r)] = eps[r].T
        m = dict(shared)
        m["xT"] = xq
        m["epsT"] = epsT
        for nm, W, b, bs in (
            ("encu", inputs["enc_Wu"][c], inputs["enc_bu"][c], SA),
            ("dec0", inputs["dec_Wu0"][c], inputs["dec_bu0"][c], 1.0),
            ("dec2", inputs["dec_Wu2"][c], inputs["dec_bu2"][c], 1.0),
            ("fin", inputs["fin_W"][c], inputs["fin_b"][c], 1.0),
        ):
            fp8 = LAYERS[nm][2]
            m["w_" + nm] = _wpack(W, FP8_NP if fp8 else BF16_NP, SW if fp8 else 1.0)
            m["b_" + nm] = _b2d(b, bs)
        in_maps.append(m)

    nc = _build_module(npad, blocks)
    res = bass_utils.run_bass_kernel_spmd(nc, in_maps, core_ids=list(range(N_CORES)))
    global LAST_RESULTS
    LAST_RESULTS = res

    out = np.empty((B, D_IN), np.float32)
    for c in range(C):
        r = rows[c]
        arr = res.results[c]["outT"]
        for off, nb in zip([sum(blocks[:i]) for i in range(len(blocks))], blocks):
            lo, hi = off, min(off + nb, len(r))
            if lo >= len(r):
                break
            seg = arr[:, KX * off : KX * (off + nb)].reshape(P, KX, nb)[:, :, : hi - lo]
            out[r[lo:hi]] = seg.transpose(2, 1, 0).reshape(hi - lo, D_IN)
    return out
